# revision 16
# baseline (speedup 1.0000x reference)
"""Multi-head self-attention Trainium2 kernel (nn_MultiHeadSA).

Sharding: data-parallel over the batch dim N across 8 NeuronCores
(one batch element per core). Each core computes its full [D, P] output,
the host just stacks the per-core results.

Math (per batch n, head h), restructured for the PE-friendly [k, q]
attention layout:

  logits[k,q] = (Wk_h x + bk)^T (Wq_h x + bq) / sqrt(D) + pos[h,k,q]
              = x^T Gh x  +  term_k[k]  +  (terms const in k -> drop
                under softmax)  + pos[h,k,q]
     Gh  = Wk_h^T Wq_h / sqrt(D)      (host-precomputed)
     term_k = x^T u_h,  u_h = Wk_h^T bq_h / sqrt(D)

  y    = Gh x                          (PE, lhsT = Gh^T)
  attn = x^T y                         (PE)
  E    = exp(attn + term_k) * exp(pos) (ScalarE exp, term_k as
                                        per-partition bias -> bf16; DVE
                                        bf16 multiply with host bf16
                                        exp(pos))
  acc  = sum_kc E_kc                   (running bf16 sum on DVE/Pool --
                                        keeps the per-chunk ones-matmuls
                                        OFF the PE critical resource)
  s    = ones[128,128]^T acc           (ONE PE matmul per q-block; the
                                        replicated stationary writes the
                                        key-sum to ALL 128 PSUM
                                        partitions -> no partition
                                        broadcast needed downstream)
  av_h = (W~_h x) E,  W~_h = Wo_h Wv_h (PE; output projection folded
                                        into V on the host)
  fin  = sum_h av_h / s_h + bo'        (DVE divide straight off PSUM +
                                        Pool add; bo' = bo + Wo bv)

The softmax normalization chain runs entirely on DVE/Pool, so the PE
instruction stream is pure matmuls: per head only y, attn, av and one
s-matmul per q-block.
"""

import numpy as np

try:
    import concourse.bass as bass
except ImportError:  # pragma: no cover
    import sys

    sys.path.insert(0, "/opt/trn_rl_repo")
    import concourse.bass as bass

from contextlib import ExitStack

import concourse.bacc as bacc
import concourse.mybir as mybir
import concourse.tile as tile

F32 = mybir.dt.float32
F32R = mybir.dt.float32r
BF16 = mybir.dt.bfloat16

N, D, P, H = 8, 256, 1024, 8
QW = 512  # q-block width (PSUM bank / fp32 moving-operand limit)

# schedule knobs
SC_BUFS = 3  # PSUM banks for the y/vt/s scratch pool
AV_BUFS = 3  # PSUM banks for av accumulation
AT_BUFS = 2  # PSUM banks for attn chunks
ACC_POOL_KCS = ()  # running-sum adds all on DVE (Pool adds stall the chain)
S_SPLIT_LAST = False  # s = ones^T acc(0..KC-2) + ones^T E_{KC-1}: the last
# chunk feeds the s-matmul directly instead of riding the serial add chain
Y_QB_OUTER = True  # y matmul emission order (qb outer: qb0 ready sooner)
EVAC_DVE_MOD = 3  # yi % EVAC_DVE_MOD == 0 -> y/vt evac on DVE, else ACT
PRELUDE_KC0 = True  # emit attn chunk kc0 before vt so exp/mul hide under it
POS_PAIR = True  # pos DMAs merged in kc pairs
LAST_SPLIT = 0  # last head q-blocks: 0=[512,512] 1=[512,256,256] 2=[512,256,128,128]
TAIL_POOL_STT = False  # Pool lacks scalar_tensor_tensor on HW (engine check)


def build_nc(h_num=H, d=D, p=P, reps=1, **knobs):
    g = globals()
    old = {k: g[k] for k in knobs}
    g.update(knobs)
    try:
        return _build_nc(h_num, d, p, reps)
    finally:
        g.update(old)


def _build_nc(h_num=H, d=D, p=P, reps=1):
    assert d % 128 == 0 and p % QW == 0 and p % 128 == 0
    IC = d // 128  # input-dim (contraction) chunks
    KC = p // 128  # key chunks
    QB = p // QW  # query blocks
    OC = d // 128  # output-dim chunks (== IC)

    nc = bacc.Bacc(None, target_bir_lowering=False)

    x_d = nc.dram_tensor("x", [d, p], F32R, kind="ExternalInput")
    pos_d = nc.dram_tensor("pos", [h_num, p, p], BF16, kind="ExternalInput")
    # gt[h] = (Wk_h^T Wq_h / sqrt(D))^T = Wq_h^T Wk_h / sqrt(D): [i', i]
    gt_d = nc.dram_tensor("gt", [h_num, d, d], F32R, kind="ExternalInput")
    # tk[r, kc, h] = x^T Wk_h^T bq_h / sqrt(D), host-precomputed and
    # pre-tiled to the SBUF layout (k = kc*128 + r) so the DMA is one
    # contiguous copy
    tk_d = nc.dram_tensor("tk", [128, KC * h_num], F32, kind="ExternalInput")
    # wt = (Wo_h Wv_h)^T per head: [d_in, h*d_out]
    wt_d = nc.dram_tensor("wt", [d, h_num * d], F32R, kind="ExternalInput")
    bo_d = nc.dram_tensor("bo", [d], F32, kind="ExternalInput")  # bo + Wo bv
    out_d = nc.dram_tensor("out", [d, p], F32, kind="ExternalOutput")
    out_r = out_d.rearrange("(c r) p -> r c p", r=128)

    with tile.TileContext(nc) as tc, ExitStack() as ctx:
        const = ctx.enter_context(tc.tile_pool(name="const", bufs=1))
        pos_pool = ctx.enter_context(
            tc.tile_pool(name="pos", bufs=8 if POS_PAIR else 10)
        )
        hbufs = ctx.enter_context(tc.tile_pool(name="hbufs", bufs=5))
        ohp = ctx.enter_context(tc.tile_pool(name="ohp", bufs=6))
        ebufs = ctx.enter_context(tc.tile_pool(name="ebufs", bufs=2))
        accp = ctx.enter_context(tc.tile_pool(name="accp", bufs=2))
        finp = ctx.enter_context(tc.tile_pool(name="finp", bufs=1))

        ps_at = ctx.enter_context(
            tc.tile_pool(name="ps_at", bufs=AT_BUFS, space="PSUM")
        )
        ps_av = ctx.enter_context(
            tc.tile_pool(name="ps_av", bufs=AV_BUFS, space="PSUM")
        )
        ps_sc = ctx.enter_context(
            tc.tile_pool(name="ps_sc", bufs=SC_BUFS, space="PSUM")
        )

        # ---- constants (head-0 slices first so compute starts early) ----
        # ones stationary for the s-matmul: memset, no DMA, no HWDGE slot
        ones_rep = const.tile([128, 128], BF16, name="ones_rep")
        nc.gpsimd.memset(ones_rep, 1.0)

        x_sb = const.tile([128, IC, p], F32R)
        x_r = x_d.rearrange("(c r) p -> r c p", r=128)
        gt_sb = const.tile([128, IC, h_num, d], F32R)
        gt_r = gt_d.rearrange("h (c r) i -> r c h i", r=128)
        wt_sb = const.tile([128, IC, h_num * d], F32R)
        wt_r = wt_d.rearrange("(c r) o -> r c o", r=128)

        # startup critical path: gt0 (ACT queue) || x qb0 (SP queue),
        # split by contraction chunk so the first y matmuls (c=0) can
        # start while the c=1 halves are still in flight
        for c in range(IC):
            nc.scalar.dma_start(out=gt_sb[:, c, 0, :], in_=gt_r[:, c, 0, :])
            nc.sync.dma_start(
                out=x_sb[:, c, bass.ts(0, QW)], in_=x_r[:, c, bass.ts(0, QW)]
            )

        # PE warm-up: a 1-element matmul on the memset constant so the
        # p-state ramp starts before the real y matmuls are ready
        warm_ps = ps_sc.tile([1, 1], F32, name="warm", tag="sc")
        nc.tensor.matmul(
            warm_ps, ones_rep[:, :1], ones_rep[:, :1], start=True, stop=True
        )

        # tk (tiny) next on ACT, then head-0 wt, then x qb1 -- ordered so
        # vt's wt0 and upper-x dependencies land just ahead of their use
        tk_sb = const.tile([128, KC, h_num], F32)
        nc.scalar.dma_start(
            out=tk_sb, in_=tk_d.rearrange("r (kc h) -> r kc h", h=h_num)
        )
        nc.sync.dma_start(
            out=wt_sb[:, :, bass.ds(0, d)], in_=wt_r[:, :, bass.ds(0, d)]
        )
        nc.sync.dma_start(
            out=x_sb[:, :, bass.ts(1, QW)], in_=x_r[:, :, bass.ts(1, QW)]
        )

        bo_sb = const.tile([128, OC], F32)
        nc.sync.dma_start(out=bo_sb, in_=bo_d.rearrange("(c r) -> r c", r=128))

        fin_sb = finp.tile([128, OC, p], F32)

        def load_head_weights(hh, eng=None):
            eng = eng or nc.sync
            eng.dma_start(out=gt_sb[:, :, hh, :], in_=gt_r[:, :, hh, :])
            eng.dma_start(
                out=wt_sb[:, :, bass.ds(hh * d, d)],
                in_=wt_r[:, :, bass.ds(hh * d, d)],
            )

        if POS_PAIR:
            pos_r = pos_d.rearrange("h (pp s r) q -> r h pp s q", s=2, r=128)
        else:
            pos_r = None

        for _rep, h in [(r0, h0) for r0 in range(reps) for h0 in range(h_num)]:
            if _rep == 0 and h + 1 < h_num:
                load_head_weights(h + 1)

            # ---- y = Gh @ x  (natural [i, q] layout) ----
            y_sb = hbufs.tile([128, IC, p], F32R)
            y_order = (
                [(qb, ic) for qb in range(QB) for ic in range(IC)]
                if Y_QB_OUTER
                else [(qb, ic) for ic in range(IC) for qb in range(QB)]
            )
            first_head = _rep == 0 and h == 0
            y_tiles = {}
            if first_head:
                # head 0: c-outer emission for qb0 so the c=0 matmuls run
                # as soon as the first DMA halves land
                for ic in range(IC):
                    y_tiles[ic] = ps_sc.tile([128, QW], F32, tag="sc", name="y")
                for c in range(IC):
                    for ic in range(IC):
                        nc.tensor.matmul(
                            y_tiles[ic],
                            gt_sb[:, c, h, bass.ts(ic, 128)],
                            x_sb[:, c, bass.ts(0, QW)],
                            start=(c == 0),
                            stop=(c == IC - 1),
                        )
                for ic in range(IC):
                    if ic % 2 == 0:
                        nc.vector.tensor_copy(
                            out=y_sb[:, ic, bass.ts(0, QW)], in_=y_tiles[ic]
                        )
                    else:
                        nc.scalar.copy(
                            out=y_sb[:, ic, bass.ts(0, QW)], in_=y_tiles[ic]
                        )
            for yi, (qb, ic) in enumerate(y_order):
                if first_head and qb == 0:
                    continue
                y_ps = ps_sc.tile([128, QW], F32, tag="sc", name="y")
                for c in range(IC):
                    nc.tensor.matmul(
                        y_ps,
                        gt_sb[:, c, h, bass.ts(ic, 128)],
                        x_sb[:, c, bass.ts(qb, QW)],
                        start=(c == 0),
                        stop=(c == IC - 1),
                    )
                if yi % EVAC_DVE_MOD == 0:
                    nc.vector.tensor_copy(
                        out=y_sb[:, ic, bass.ts(qb, QW)], in_=y_ps
                    )
                else:
                    nc.scalar.copy(
                        out=y_sb[:, ic, bass.ts(qb, QW)], in_=y_ps
                    )

            # pos chunks for this head, prefetched during the y/vt phase
            if POS_PAIR:
                pos_t = [
                    pos_pool.tile([128, 2, p], BF16, name="pos", tag="pos")
                    for _ in range(KC // 2)
                ]
                for pp in range(KC // 2):
                    nc.sync.dma_start(out=pos_t[pp], in_=pos_r[:, h, pp, :, :])

                def pos_chunk(kc):
                    return pos_t[kc // 2][:, kc % 2, :]
            else:
                pos_t = [
                    pos_pool.tile([128, p], BF16, name="pos", tag="pos")
                    for _ in range(KC)
                ]
                for kc in range(KC):
                    nc.sync.dma_start(
                        out=pos_t[kc], in_=pos_d[h, bass.ts(kc, 128), :]
                    )

                def pos_chunk(kc):
                    return pos_t[kc]

            last_h = _rep == reps - 1 and h == h_num - 1
            if last_h and LAST_SPLIT:
                # narrower q-blocks for the final head: the exposed
                # normalize/store tail after the last matmul scales with
                # the block width
                if LAST_SPLIT == 1:
                    widths = [QW, QW // 2, QW // 2]
                elif LAST_SPLIT == 3:
                    widths = [QW, 3 * QW // 4, QW // 4]
                else:
                    widths = [QW, QW // 2, QW // 4, QW // 4]
                qblocks = []
                q = 0
                for wq in widths:
                    qblocks.append((q, wq))
                    q += wq
            else:
                qblocks = [(qb * QW, QW) for qb in range(QB)]

            def emit_attn_chunk(kc, e_sb, acc, q0, w):
                at_ps = ps_at.tile([128, w], F32, name="at_ps", tag="at_ps")
                for c in range(IC):
                    nc.tensor.matmul(
                        at_ps,
                        x_sb[:, c, bass.ts(kc, 128)],
                        y_sb[:, c, bass.ds(q0, w)],
                        start=(c == 0),
                        stop=(c == IC - 1),
                    )
                # E = exp(attn + term_k) * exp(pos)
                nc.scalar.activation(
                    out=e_sb[:, kc, :],
                    in_=at_ps,
                    func=mybir.ActivationFunctionType.Exp,
                    bias=tk_sb[:, kc, h : h + 1],
                    scale=1.0,
                )
                nc.vector.tensor_mul(
                    e_sb[:, kc, :],
                    e_sb[:, kc, :],
                    pos_chunk(kc)[:, bass.ds(q0, w)],
                )
                # running key-sum: acc holds sum of chunks 0..kc (bf16);
                # with S_SPLIT_LAST the final chunk is contracted by the
                # s-matmul itself, keeping it off the serial add chain
                acc_last = KC - 2 if S_SPLIT_LAST else KC - 1
                if kc == 1:
                    nc.vector.tensor_add(acc, e_sb[:, 0, :], e_sb[:, 1, :])
                elif 1 < kc <= acc_last:
                    eng = nc.gpsimd if kc in ACC_POOL_KCS else nc.vector
                    eng.tensor_add(acc, acc, e_sb[:, kc, :])

            pre_tiles = None
            if PRELUDE_KC0:
                # first q-block's tiles + attn chunk kc0, emitted before vt
                # so the exp/mul handoff latency hides under the vt matmuls
                q0p, wp = qblocks[0]
                e_sb_p = ebufs.tile([128, KC, wp], BF16, name="e_sb", tag="e_sb")
                acc_p = accp.tile([128, wp], BF16, name="acc", tag="acc")
                av_ps_p = [
                    ps_av.tile([128, wp], F32, tag="av", name=f"av{dc}")
                    for dc in range(IC)
                ]
                emit_attn_chunk(0, e_sb_p, acc_p, q0p, wp)
                pre_tiles = (e_sb_p, acc_p, av_ps_p)

            # ---- vT = ((Wo_h Wv_h) x)^T  ([p, o] layout, bf16) ----
            # two p-chunks share one PSUM bank -> one wide evacuation
            vt_sb = hbufs.tile([128, KC, d], BF16)
            for pp in range(KC // 2):
                vt_ps = ps_sc.tile([128, 2, d], F32, tag="sc", name="vt")
                for sub in range(2):
                    pc = 2 * pp + sub
                    for c in range(IC):
                        nc.tensor.matmul(
                            vt_ps[:, sub, :],
                            x_sb[:, c, bass.ts(pc, 128)],
                            wt_sb[:, c, bass.ds(h * d, d)],
                            start=(c == 0),
                            stop=(c == IC - 1),
                        )
                if pp % EVAC_DVE_MOD == 0:
                    nc.vector.tensor_copy(
                        out=vt_sb[:, bass.ts(pp, 2), :], in_=vt_ps
                    )
                else:
                    nc.scalar.copy(
                        out=vt_sb[:, bass.ts(pp, 2), :], in_=vt_ps
                    )

            def alloc_qb_tiles(w):
                e_sb = ebufs.tile([128, KC, w], BF16, name="e_sb", tag="e_sb")
                acc = accp.tile([128, w], BF16, name="acc", tag="acc")
                av_ps = [
                    ps_av.tile([128, w], F32, tag="av", name=f"av{dc}")
                    for dc in range(IC)
                ]
                return e_sb, acc, av_ps

            nxt_tiles = pre_tiles
            for qbi, (q0, w) in enumerate(qblocks):
                if nxt_tiles is not None:
                    e_sb, acc, av_ps = nxt_tiles
                    kc_start = 1
                else:
                    e_sb, acc, av_ps = alloc_qb_tiles(w)
                    kc_start = 0
                nxt_tiles = None

                def emit_av(kc):
                    # (W~ x) E accumulation
                    for dc in range(IC):
                        nc.tensor.matmul(
                            av_ps[dc],
                            vt_sb[:, kc, bass.ts(dc, 128)],
                            e_sb[:, kc, :],
                            start=(kc == 0),
                            stop=(kc == KC - 1),
                        )

                for kc in range(kc_start, KC):
                    emit_attn_chunk(kc, e_sb, acc, q0, w)
                    # av runs one chunk behind so the PE never waits on the
                    # exp handoff
                    if kc > 0:
                        emit_av(kc - 1)
                emit_av(KC - 1)

                # softmax denominator: ONE matmul per q-block over the
                # pre-accumulated chunk sum; the [128,128] ones stationary
                # replicates s across all PSUM partitions. Allocated from
                # the av pool so the y/vt scratch pool never blocks on the
                # tail divides.
                s_ps = ps_av.tile([128, w], F32, tag="av", name="s_rep")
                if S_SPLIT_LAST:
                    nc.tensor.matmul(s_ps, ones_rep, acc, start=True, stop=False)
                    nc.tensor.matmul(
                        s_ps, ones_rep, e_sb[:, KC - 1, :], start=False, stop=True
                    )
                else:
                    nc.tensor.matmul(s_ps, ones_rep, acc, start=True, stop=True)

                # prelude for the NEXT q-block: its first attn chunk runs
                # here so its exp/mul chain hides under the s-matmuls and
                # the next block's early attn chunks. Emitted AFTER s so
                # the av-pool slot rotation stays clean.
                if qbi + 1 < len(qblocks):
                    q0n, wn = qblocks[qbi + 1]
                    nxt_tiles = alloc_qb_tiles(wn)
                    emit_attn_chunk(0, nxt_tiles[0], nxt_tiles[1], q0n, wn)

                # normalization + head accumulation: reciprocal of the
                # replicated s straight off PSUM (no partition broadcast
                # needed), then PSUM-av times SBUF-1/s on DVE.
                rr_sb = ohp.tile([128, w], F32, name="rr_sb", tag="rr_sb")
                nc.vector.reciprocal(out=rr_sb, in_=s_ps)
                last = last_h
                for dc in range(IC):
                    dst = fin_sb[:, dc, bass.ds(q0, w)]
                    if h == 0 and _rep == 0:
                        nc.vector.tensor_mul(dst, av_ps[dc], rr_sb)
                    else:
                        tmp = ohp.tile([128, w], F32, name="tmp", tag="tmp")
                        nc.vector.tensor_mul(tmp, av_ps[dc], rr_sb)
                        if last:
                            eng = nc.gpsimd if TAIL_POOL_STT else nc.vector
                            eng.scalar_tensor_tensor(
                                out=dst,
                                in0=tmp,
                                scalar=bo_sb[:, dc : dc + 1],
                                in1=dst,
                                op0=mybir.AluOpType.add,
                                op1=mybir.AluOpType.add,
                            )
                            if qbi == len(qblocks) - 1:
                                # final block: per-dc store directly after
                                # its stt so it overlaps the next dc's work
                                nc.sync.dma_start(
                                    out=out_d[bass.ts(dc, 128), bass.ds(q0, w)],
                                    in_=dst,
                                )
                        else:
                            nc.gpsimd.tensor_add(dst, dst, tmp)
                if last and qbi != len(qblocks) - 1:
                    # single merged store per q-block (one HWDGE slot)
                    nc.sync.dma_start(
                        out=out_r[:, :, bass.ds(q0, w)],
                        in_=fin_sb[:, :, bass.ds(q0, w)],
                    )

    nc.finalize()
    return nc


def prep_weights(Wk, bk, Wq, bq, Wv, bv, Wo, bo, h_num=H, d=D):
    """Host-side weight transformation (float64 accumulate)."""
    Wk = np.asarray(Wk, np.float64).reshape(h_num, d, d)
    Wq = np.asarray(Wq, np.float64).reshape(h_num, d, d)
    bq = np.asarray(bq, np.float64).reshape(h_num, d)
    Wv = np.asarray(Wv, np.float64).reshape(h_num, d, d)
    bv = np.asarray(bv, np.float64)
    Wo = np.asarray(Wo, np.float64)
    bo = np.asarray(bo, np.float64)
    s = 1.0 / np.sqrt(d)

    # lhsT for the y-matmul is Gh^T = Wq_h^T Wk_h * s
    gt = np.einsum("hdi,hdj->hij", Wq, Wk) * s
    u = np.einsum("hdi,hd->ih", Wk, bq) * s  # u[i, h]
    # W~_h = Wo_h @ Wv_h; lhsT layout wt[d_in, h*d_out] = W~_h^T stacked
    Wo_h = Wo.reshape(d, h_num, d).transpose(1, 0, 2)  # [h, d_out, d]
    wtil = np.einsum("hod,hdi->hoi", Wo_h, Wv)  # [h, d_out, d_in]
    wt = np.concatenate([wtil[hh].T for hh in range(h_num)], axis=1)  # [d_in, h*d_out]
    bo2 = bo + Wo @ bv
    return (
        gt.astype(np.float32),
        u.astype(np.float64),
        np.ascontiguousarray(wt).astype(np.float32),
        bo2.astype(np.float32),
    )


def prep_pos(pos_mat):
    """Host-side: exp(pos) in bf16 (shared across all cores)."""
    import ml_dtypes

    return np.exp(np.asarray(pos_mat, np.float64)[0]).astype(ml_dtypes.bfloat16)


def make_in_maps(inputs):
    """Per-core input maps from the full reference inputs dict."""
    gt, u, wt, bo2 = prep_weights(
        inputs["Wk"], inputs["bk"], inputs["Wq"], inputs["bq"],
        inputs["Wv"], inputs["bv"], inputs["Wo"], inputs["bo"],
    )
    pos = np.ascontiguousarray(prep_pos(inputs["pos_mat"]))
    x_all = np.asarray(inputs["inputs"], np.float32)
    # tk[n, k, h] = x_n^T u (host, f64) — the per-key exp bias,
    # pre-tiled to [128 partitions, kc*h] (k = kc*128 + r)
    tk_all = np.einsum(
        "ndp,dh->nph", np.asarray(x_all, np.float64), u
    ).astype(np.float32)
    KC = P // 128
    tk_tiled = [
        np.ascontiguousarray(
            tk_all[n].reshape(KC, 128, H).transpose(1, 0, 2).reshape(128, KC * H)
        )
        for n in range(N)
    ]
    return [
        dict(x=x_all[n], pos=pos, gt=gt, tk=tk_tiled[n], wt=wt, bo=bo2)
        for n in range(N)
    ]


_NC_CACHE = {}


def _get_nc():
    if "nc" not in _NC_CACHE:
        _NC_CACHE["nc"] = build_nc()
    return _NC_CACHE["nc"]


def kernel(inputs, pos_mat, Wk, bk, Wq, bq, Wv, bv, Wo, bo, **run_kwargs):
    from concourse.bass_utils import run_bass_kernel_spmd

    in_maps = make_in_maps(
        dict(inputs=inputs, pos_mat=pos_mat, Wk=Wk, bk=bk, Wq=Wq, bq=bq,
             Wv=Wv, bv=bv, Wo=Wo, bo=bo)
    )
    nc = _get_nc()
    res = run_bass_kernel_spmd(nc, in_maps, core_ids=list(range(N)), **run_kwargs)
    out = np.stack([res.results[n]["out"] for n in range(N)])
    _NC_CACHE["last_result"] = res
    return out.astype(np.float32)
